# revision 9
# baseline (speedup 1.0000x reference)
"""AdaptiveSinLayer kernel for 8 TRN2 NeuronCores.

Computation: out[t] = sin(OMEGA * (x[t] @ weight[indices[t]] + bias)),
x: [1024, 256, 256] f32, weight: [1024, 256, 256] f32, indices: [1024] i64.

Strategy: data-parallel over the leading tile dim (128 tiles/core). The
weight table (pre-scaled by OMEGA, bf16, [I-chunk, O] row-blocked) is
replicated into every core's DRAM; each tile's routed weight matrix is
fetched on-device by a dynamically-addressed DMA: the channel id is
reg_load-ed into an engine register and used as a DynSlice offset into
the table (HWDGE dynamic descriptor generation). Gathers alternate
between the SP and ACT HWDGE rings.

Per tile t, PSUM layout psum[o', 256m+p] = OMEGA*(x[t] @ w[c])[p, 128m+o']
built from 4 bf16 MMs + 2 K=1 bias MMs. Tiles are processed in quads
(q=4 tiles = one [128, 2048] f32 PSUM span = 4 banks) so each pointwise
pass is one instruction over 2048 columns:
  u   = psum/2pi + C        (magic-const f32 RN add rounds to integer)
  s   = (u - C) * -2pi      (= -2pi*round(psum/2pi))
  arg = s + psum            (in [-pi, pi])
  out = Sin(arg)            (ACT; HW Sin valid only on [-pi, pi])
The engine for u/s/arg is configurable per-quad (cycled) to balance
ACT/DVE/GPSIMD occupancy; Sin is ACT-only.
"""
import numpy as np
import ml_dtypes
from contextlib import ExitStack

from concourse import bass, bacc, mybir, tile
from concourse.bass import make_scalar_value, RegisterHandles
from concourse.bass_utils import run_bass_kernel_spmd

N_CORES = 8
T, P, I, O, N_CH = 1024, 256, 256, 256, 1024
T_SH = T // N_CORES
OMEGA = 30.0
PI = float(np.pi)
TWO_PI = float(2 * np.pi)
INV_2PI = float(1.0 / (2 * np.pi))
C_MAGIC = float(1.5 * 2**23)

BF16 = mybir.dt.bfloat16
F32 = mybir.dt.float32
I32 = mybir.dt.int32

G = 8   # tiles per x/out DMA group
Q = 2   # tiles per pointwise pair (PSUM span = Q banks)


def build_nc(repeat=1, w_bufs=16, x_bufs=4, o_bufs=3, mm_bufs=9,
             out_engine="scalar", gather_engines=("sync",),
             x_engine="sync",
             u_engines=("scalar",), s_engines=("vector",),
             arg_engines=("vector",), g=None, q=None, out_bf16=True,
             mode="chain", ablate=None):
    ablate = (set() if ablate is None
              else {ablate} if isinstance(ablate, str) else set(ablate))
    G_ = g if g is not None else G
    Q_ = q if q is not None else Q
    N_G_ = T_SH // G_
    nc = bacc.Bacc(None, target_bir_lowering=False)
    # Device layouts (per core):
    #  xT:  [T_SH, 128, 512]  xT[t, i', k*256+p] = x[t, p, 128k+i']          bf16
    #  wt:  [N_CH, 128, 512]  wt[c, i', k*256+o] = OMEGA*w[c, 128k+i', o]   bf16
    #  bv:  [1, 512]          cols 0-255: OMEGA*b (lhsT), 256-511: ones      bf16
    #  idx: [1, T_SH]         raw channel ids                               i32
    #  out: [T_SH, 128, 2, P] out[t, o', m, p] = result[t, p, 128m+o']       f32
    xT = nc.declare_dram_parameter("xT", [T_SH, 128, 512], BF16, isOutput=False)
    wt = nc.declare_dram_parameter("wt", [N_CH, 128, 512], BF16, isOutput=False)
    bo = nc.declare_dram_parameter("bo", [1, 512], BF16, isOutput=False)
    idx = nc.declare_dram_parameter("idx", [1, T_SH], I32, isOutput=False)
    out_dt = BF16 if out_bf16 else F32
    out = nc.declare_dram_parameter("out", [T_SH, 128, 2, P], out_dt, isOutput=True)

    with tile.TileContext(nc) as tc, ExitStack() as ctx:
        const_pool = ctx.enter_context(tc.tile_pool(name="const", bufs=1))
        x_pool = ctx.enter_context(tc.tile_pool(name="x", bufs=x_bufs))
        w_pool = ctx.enter_context(tc.tile_pool(name="w", bufs=w_bufs))
        mm_pool = ctx.enter_context(tc.tile_pool(name="mm", bufs=mm_bufs))
        o_pool = ctx.enter_context(tc.tile_pool(name="o", bufs=o_bufs))
        psum_pool = ctx.enter_context(
            tc.tile_pool(name="psum", bufs=8 // Q_, space="PSUM")
        )

        idx_sb = const_pool.tile([1, T_SH], I32)
        nc.sync.dma_start(idx_sb[:], idx[:])
        bo_sb = const_pool.tile([1, 512], BF16)
        nc.sync.dma_start(bo_sb[:], bo[:])
        c_magic = const_pool.tile([128, 1], F32)
        nc.gpsimd.memset(c_magic[:], C_MAGIC)
        wb0 = None
        if "nogather" in ablate:
            wb0 = const_pool.tile([128, 512], BF16)
            nc.sync.dma_start(wb0[:], wt[0, :, :])
        xb0 = None
        if "noxdma" in ablate:
            xb0 = const_pool.tile([128, G_, 512], BF16)
            nc.sync.dma_start(xb0[:], xT[0:G_].rearrange("t i f -> i t f"))

        all_engines = set(gather_engines)
        regs = {e: getattr(nc, e).alloc_register(f"gidx_{e}") for e in all_engines}

        def pointwise(psum, ob, qslice, qidx):
            if "nopointwise" in ablate:
                nc.scalar.activation(
                    ob[:, qslice, :, :], psum[:],
                    mybir.ActivationFunctionType.Sin, bias=0.0, scale=1.0,
                )
                return
            u_name = u_engines[qidx % len(u_engines)]
            s_name = s_engines[qidx % len(s_engines)]
            a_name = arg_engines[qidx % len(arg_engines)]
            n = psum.shape[1]
            if mode == "copyfirst":
                # drain PSUM once via ACT; the whole reduction chain runs
                # SBUF-side on DVE (no PSUM re-reads, no inter-op sems)
                z_sb = mm_pool.tile([128, n], F32, tag="z")
                nc.scalar.activation(
                    z_sb[:], psum[:], mybir.ActivationFunctionType.Copy,
                    bias=0.0, scale=1.0,
                )
                u_sb = mm_pool.tile([128, n], F32, tag="u")
                nc.vector.tensor_scalar(
                    u_sb[:], z_sb[:], INV_2PI, C_MAGIC,
                    mybir.AluOpType.mult, mybir.AluOpType.add,
                )
                s_sb = mm_pool.tile([128, n], F32, tag="s")
                nc.vector.tensor_scalar(
                    s_sb[:], u_sb[:], C_MAGIC, -TWO_PI,
                    mybir.AluOpType.subtract, mybir.AluOpType.mult,
                )
                arg_sb = mm_pool.tile([128, n], F32, tag="arg")
                nc.vector.tensor_tensor(
                    arg_sb[:], s_sb[:], z_sb[:], mybir.AluOpType.add
                )
                nc.scalar.activation(
                    ob[:, qslice, :, :], arg_sb[:],
                    mybir.ActivationFunctionType.Sin, bias=0.0, scale=1.0,
                )
                return
            u_sb = mm_pool.tile([128, n], F32, tag="u")
            if u_name == "scalar":
                nc.scalar.activation(
                    u_sb[:], psum[:], mybir.ActivationFunctionType.Identity,
                    bias=c_magic[:], scale=INV_2PI,
                )
            else:
                getattr(nc, u_name).tensor_scalar(
                    u_sb[:], psum[:], INV_2PI, C_MAGIC,
                    mybir.AluOpType.mult, mybir.AluOpType.add,
                )
            s_sb = mm_pool.tile([128, n], F32, tag="s")
            getattr(nc, s_name).tensor_scalar(
                s_sb[:], u_sb[:], C_MAGIC, -TWO_PI,
                mybir.AluOpType.subtract, mybir.AluOpType.mult,
            )
            arg_sb = mm_pool.tile([128, n], F32, tag="arg")
            getattr(nc, a_name).tensor_tensor(
                arg_sb[:], s_sb[:], psum[:], mybir.AluOpType.add
            )
            nc.scalar.activation(
                ob[:, qslice, :, :], arg_sb[:],
                mybir.ActivationFunctionType.Sin, bias=0.0, scale=1.0,
            )

        def group_body(gi):
            t0 = gi * G_
            if "noxdma" in ablate:
                xb = xb0
            else:
                xb = x_pool.tile([128, G_, 512], BF16)
                getattr(nc, x_engine).dma_start(
                    xb[:], xT[t0 : t0 + G_].rearrange("t i f -> i t f")
                )

            wbs = []
            for j in range(G_):
                if "nogather" in ablate:
                    wbs.append(wb0)
                    continue
                wb = w_pool.tile([128, 512], BF16, tag="wb")
                eng_name = gather_engines[(t0 + j) % len(gather_engines)]
                eng = getattr(nc, eng_name)
                if "static" in ablate:
                    eng.dma_start(wb[:], wt[(t0 + j) % N_CH, :, :])
                else:
                    r = regs[eng_name]
                    eng.reg_load(r, idx_sb[0:1, t0 + j : t0 + j + 1])
                    off = make_scalar_value(
                        RegisterHandles(r), min_val=0, max_val=N_CH - 1
                    )
                    eng.dma_start(wb[:], wt[bass.ds(off, 1), :, :])
                wbs.append(wb)

            ob = o_pool.tile([128, G_, 2, P], out_dt)
            for qi in range(G_ // Q_):
                psum = psum_pool.tile([128, Q_ * 512], F32)
                for jj in range(Q_):
                    j = qi * Q_ + jj
                    wb = wbs[j]
                    n_k = 1 if "halfmm" in ablate else 2
                    for m in range(2):
                        col = 512 * jj + 256 * m
                        for k in range(n_k):
                            nc.tensor.matmul(
                                psum[:, col : col + 256],
                                wb[:, 256 * k + 128 * m : 256 * k + 128 * (m + 1)],
                                xb[:, j, 256 * k : 256 * (k + 1)],
                                start=(k == 0),
                                stop=("halfmm" in ablate and k == n_k - 1),
                            )
                        if "halfmm" not in ablate:
                            # bias via K=1 bf16 matmul accumulate
                            nc.tensor.matmul(
                                psum[:, col : col + 256],
                                bo_sb[0:1, 128 * m : 128 * (m + 1)],
                                bo_sb[0:1, 256:512],
                                start=False,
                                stop=True,
                            )
                qidx = gi * (G_ // Q_) + qi
                pointwise(psum, ob, slice(qi * Q_, (qi + 1) * Q_), qidx)
            if "nooutdma" not in ablate or gi == 0:
                getattr(nc, out_engine).dma_start(
                    out[t0 : t0 + G_].rearrange("t i m p -> i t m p"), ob[:]
                )

        def full_body(_iv=None):
            for gi in range(N_G_):
                group_body(gi)

        if repeat == 1:
            full_body()
        else:
            # benchmarking: run the whole per-core program `repeat` times
            with tc.For_i(0, repeat, 1):
                full_body()

    nc.compile()
    return nc


_NC = None


def _get_nc():
    global _NC
    if _NC is None:
        _NC = build_nc()
    return _NC


def make_in_maps(x, weight, bias, indices):
    """Host-side shard/layout prep. Returns in_maps for run_bass_kernel_spmd."""
    x = np.asarray(x, dtype=np.float32)
    weight = np.asarray(weight, dtype=np.float32)
    bias = np.asarray(bias, dtype=np.float32).reshape(O)
    indices = np.asarray(indices).astype(np.int64)

    # wt[c, i', k*256+o] = OMEGA*w[c, 128k+i', o]
    wt_h = np.ascontiguousarray(
        (OMEGA * weight).reshape(N_CH, 2, 128, O).transpose(0, 2, 1, 3)
    ).astype(ml_dtypes.bfloat16).reshape(N_CH, 128, 512)
    bo_h = np.concatenate(
        [(OMEGA * bias).reshape(256), np.ones(256, np.float32)]
    ).astype(ml_dtypes.bfloat16).reshape(1, 512)

    in_maps = []
    for c in range(N_CORES):
        xs = x[c * T_SH : (c + 1) * T_SH]  # [T_SH, P, I]
        xT_h = (
            np.ascontiguousarray(xs.reshape(T_SH, P, 2, 128).transpose(0, 3, 2, 1))
            .astype(ml_dtypes.bfloat16)
            .reshape(T_SH, 128, 512)
        )
        idx_h = indices[c * T_SH : (c + 1) * T_SH].astype(np.int32).reshape(1, T_SH)
        in_maps.append({"xT": xT_h, "wt": wt_h, "bo": bo_h, "idx": idx_h})
    return in_maps


def unshard(results):
    """results: list of per-core dicts with 'out' [T_SH, 128, 2, P] -> [T, P, O]."""
    outs = []
    for r in results:
        o = np.asarray(r["out"]).astype(np.float32)  # [T_SH, 128(o'), 2(m), P]
        outs.append(o.transpose(0, 3, 2, 1).reshape(T_SH, P, O))
    return np.concatenate(outs, axis=0)


def kernel(x, weight, bias, indices):
    nc = _get_nc()
    in_maps = make_in_maps(x, weight, bias, indices)
    try:
        res = run_bass_kernel_spmd(nc, in_maps, core_ids=list(range(N_CORES)))
    except ModuleNotFoundError:
        # BASS_TRACE set but the axon NTFF hook module is absent: run untraced.
        import os

        os.environ["BASS_NEVER_TRACE"] = "1"
        res = run_bass_kernel_spmd(nc, in_maps, core_ids=list(range(N_CORES)))
    return unshard(res.results)


if __name__ == "__main__":
    rng = np.random.default_rng(0)
    bound = float(np.sqrt(6.0 / I) / OMEGA)
    x = rng.standard_normal((T, P, I), dtype=np.float32)
    w = rng.uniform(-bound, bound, size=(N_CH, I, O)).astype(np.float32)
    b = rng.uniform(-bound, bound, size=(1, 1, O)).astype(np.float32)
    idx = rng.integers(0, N_CH, size=(T,), dtype=np.int64)
    got = kernel(x, w, b, idx)
    wg = w[idx]
    ref = np.sin(OMEGA * (np.einsum("tpi,tio->tpo", x, wg) + b))
    rel = np.linalg.norm(got - ref) / np.linalg.norm(ref)
    print("Relative error:", rel)
